# revision 21
# baseline (speedup 1.0000x reference)
"""Trainium2 Bass kernel for nn_BCE_Loss (retrieval_knn).

Distributed strategy (8 NeuronCores, SPMD, AllGather):
  - Each core receives ONLY its own 1024 rows of the batch.
  - Phase 1 (per core): L2-normalize own rows (f32), scale by S=64, cast
    bf16, transpose via PE, cast fp8e4 on the PSUM->SBUF copy into an
    xT chunk [128, 4 dtiles, 1024 rows]; DMA the chunk to DRAM.
  - AllGather (NRT collective) shares the 8 fp8 chunks so every core
    holds the full xT [512, 8192] in fp8.
  - Phase 2 (per core): fp8 DoubleRow matmuls (256-deep contraction per
    instruction) compute the [1024, 8192] similarity stripe tile-by-tile
    into PSUM f32 (values = S^2 * cos); ACT evacuates each [128, 1024]
    tile with a fused magic-number rounding (t = v*(2^24/S^2) + 1.5*2^36
    snaps cos*2^24 to the 2^13 grid); DVE packs the local column exactly
    (p = (t - BIG) + iota) and takes top-8 per 1024-column block; per
    row-tile the 64 packed candidates merge via 3 x (max8 +
    match_replace) into sorted top-24 (value and global column share one
    f32). Row-tile-major order so each row-tile's merge overlaps the
    next row-tile's scans.
  - No diagonal masking: the self-match is always the global top-1
    (cos=1 vs <=0.3), so the host simply drops slot 0 and uses slots
    1..k (k <= 23).
  - Host: decode (value, column), gather labels, compute the BCE mean.
"""

from contextlib import ExitStack

import numpy as np

import concourse.bass as bass
import concourse.mybir as mybir
import concourse.tile as tile
from concourse.bass import ts
from concourse.bass_utils import run_bass_kernel_spmd
from concourse.masks import make_identity
from concourse.vector_clock import ScopedClock, VectorClock

F32 = mybir.dt.float32
BF16 = mybir.dt.bfloat16
FP8 = mybir.dt.float8e4
U32 = mybir.dt.uint32
I32 = mybir.dt.int32
AF = mybir.ActivationFunctionType
ALU = mybir.AluOpType

B, D = 8192, 512
M = 8              # cores
BL = B // M        # 1024 rows per core
NRT = BL // 128    # 8 row tiles per core
NBLK = 8           # 1024-column scan blocks
S = 64.0           # fp8 pre-scale; PSUM values are S^2 * cos
EVAC_SCALE = 16777216.0 / (S * S)   # -> cos * 2^24
BIGMAGIC = 103079215104.0  # 1.5 * 2^36: rounds cos*2^24 to multiples of 2^13
NEG = -3.0e38


# ---------------------------------------------------------------------------
# Environment workarounds: this container's walrus accepts at most ONE sem
# wait per instruction, and its runtime crashes on the explicit EventSemaphore
# butterfly barrier TileContext emits at its tail.
# ---------------------------------------------------------------------------

def _patched_drain_and_barrier(self, tick_clock, wait_clock):
    nc = self.nc
    vc = tick_clock.global_clock
    n = len(vc)
    for p in range(n):
        t = vc[p]
        if t > 0:
            pvc = VectorClock([0] * n)
            pvc.require_at_least(p, t)
            nop = nc.sync.nop()
            wait_clock.add_sem_waits(nop.ins, ScopedClock({None: pvc}))
    nc.sync.drain()
    nc._nrt_pseudo_barrier()
    assert self.sems is not None
    popped = nc._tile_sem_poison_stack.pop()
    assert popped is self._sem_poison
    nc.clear_and_free_semaphores(list(self.sems.allocated().values()))
    nc._nrt_pseudo_barrier()


tile.TileContext._drain_and_barrier = _patched_drain_and_barrier


def _split_multi_waits(nc):
    import bass_rust

    for f in nc.m.functions:
        for bb in f.blocks:
            out = []
            changed = False
            for ins in bb.instructions:
                si = ins.sync_info
                waits = list(si.on_wait) if si is not None else []
                if len(waits) > 1:
                    changed = True
                    for w in waits[:-1]:
                        nop = mybir.InstNoOp(
                            name=f"I-wsplit-{nc.next_id()}", ins=[], outs=[]
                        )
                        nop.engine = ins.engine
                        nop.sync_info = bass_rust.SyncInfo(on_wait=[w], on_update=[])
                        out.append(nop)
                    ins.sync_info = bass_rust.SyncInfo(
                        on_wait=[waits[-1]], on_update=list(si.on_update)
                    )
                out.append(ins)
            if changed:
                bb.instructions = out


# ---------------------------------------------------------------------------
# Kernel build
# ---------------------------------------------------------------------------

def build_nc(repeat=1, skip_cc=False):
    nc = bass.Bass(num_devices=M)
    x = nc.declare_dram_parameter("x", [BL, D], F32, isOutput=False)
    out = nc.declare_dram_parameter("out", [BL, 24], F32, isOutput=True)
    # per-half staging: half h covers chunk columns (own rows) h*512..h*512+512,
    # laid out d4-major: cc_in[h][:, d4*512 + c] = xT[d4*128+p, h*512+c]
    cc_in = [nc.dram_tensor(f"cc_in{h}", [128, 2048], FP8) for h in range(2)]
    cc_out = [nc.dram_tensor(f"cc_out{h}", [M * 128, 2048], FP8,
                             addr_space="Shared") for h in range(2)]
    for _rep in range(repeat):
        _build_body(nc, x, out, cc_in, cc_out, skip_cc)
    _split_multi_waits(nc)
    return nc


def _phase1_half(nc, tc, octx, x, cc_in_h, half):
    """Normalize 4 of the 8 own row-tiles, transpose, cast fp8, stage the
    half-chunk to DRAM so its AllGather can fly while the other half of
    phase 1 computes."""
    cpool = octx.enter_context(tc.tile_pool(name=f"c1{half}", bufs=1))
    ident_bf = cpool.tile([128, 128], BF16)
    make_identity(nc, ident_bf[:])

    xo_pool = octx.enter_context(tc.tile_pool(name=f"xo{half}", bufs=1))
    xt_own = xo_pool.tile([128, 4, 512], FP8)

    ld = octx.enter_context(tc.tile_pool(name=f"ld{half}", bufs=3))
    sm = octx.enter_context(tc.tile_pool(name=f"sm{half}", bufs=4))
    tpp = octx.enter_context(
        tc.tile_pool(name=f"tp{half}", bufs=2, space="PSUM"))
    for rt4 in range(4):
        rt = half * 4 + rt4
        xtile = ld.tile([128, D], F32, tag="xtile")
        nc.sync.dma_start(xtile[:], x[ts(rt, 128), :])
        sq = ld.tile([128, D], F32, tag="sq")
        ss = sm.tile([128, 1], F32, tag="ss")
        nc.scalar.activation(sq[:], xtile[:], AF.Square, accum_out=ss[:])
        nrm = sm.tile([128, 1], F32, tag="nrm")
        # nrm = sqrt(ss / S^2) = |x| / S
        nc.scalar.activation(nrm[:], ss[:], AF.Sqrt, scale=1.0 / (S * S))
        rcp = sm.tile([128, 1], F32, tag="rcp")
        nc.vector.reciprocal(rcp[:], nrm[:])       # S / |x|
        xb = ld.tile([128, D], BF16, tag="xb")
        nc.vector.tensor_scalar_mul(xb[:], xtile[:], rcp[:])
        tps = tpp.tile([128, 512], BF16, tag="tp")
        for d4 in range(4):
            nc.tensor.transpose(tps[:, ts(d4, 128)], xb[:, ts(d4, 128)],
                                ident_bf[:])
        # PSUM bf16 -> SBUF fp8 cast while laying out the half chunk
        nc.scalar.copy(
            xt_own[:, :, ts(rt4, 128)],
            tps[:].rearrange("p (d c) -> p d c", c=128),
        )
    # stage the half chunk out for its AllGather (4 x 64KB)
    for d4 in range(4):
        nc.sync.dma_start(cc_in_h[:, ts(d4, 512)], xt_own[:, d4, :])


def _phase2(nc, tc, octx, out, cc_in, cc_out):
    cpool = octx.enter_context(tc.tile_pool(name="c2", bufs=1))
    # off[p, i] = 1024 * (i // 8): scan-block base for candidate slot i
    off_i = cpool.tile([128, 64], I32)
    nc.gpsimd.iota(off_i[:], pattern=[[1024, 8], [0, 8]], base=0,
                   channel_multiplier=0)
    off_f = cpool.tile([128, 64], F32)
    nc.scalar.copy(off_f[:], off_i[:])
    iota_i = cpool.tile([128, 1024], I32)
    nc.gpsimd.iota(iota_i[:], pattern=[[1, 1024]], base=0,
                   channel_multiplier=0)
    iota_f = cpool.tile([128, 1024], F32)
    nc.scalar.copy(iota_f[:], iota_i[:])

    xt_pool = octx.enter_context(tc.tile_pool(name="xt", bufs=1))
    xt_lhs = xt_pool.tile([128, 4, 1024], FP8, tag="xt_lhs", name="xt_lhs")
    xt = [
        xt_pool.tile([128, 4, 1024], FP8, tag=f"xt_{a}", name=f"xt_{a}")
        for a in range(NBLK)
    ]
    # own chunk (lhsT) straight from the staging bounce; gathered chunks
    # (rhs) from the collective output.
    # chunk-major fetch order: both halves of xt_lhs first, then both
    # halves of each chunk in scan order, so block a's matmuls (which
    # need both column halves) unblock after ~(a+2)*512KB instead of
    # waiting for the entire half-0 sweep.
    for h in range(2):
        nc.sync.dma_start(
            xt_lhs[:, :, h * 512:h * 512 + 512],
            cc_in[h][:, :].rearrange("p (d c) -> p d c", c=512))
    for a in range(NBLK):
        for h in range(2):
            nc.sync.dma_start(
                xt[a][:, :, h * 512:h * 512 + 512],
                cc_out[h][ts(a, 128), :].rearrange("p (d c) -> p d c",
                                                   c=512))

    mm = octx.enter_context(tc.tile_pool(name="mm", bufs=4, space="PSUM"))
    sb = octx.enter_context(tc.tile_pool(name="sb", bufs=6))
    pk = octx.enter_context(tc.tile_pool(name="pk", bufs=6))
    cand = octx.enter_context(tc.tile_pool(name="cand", bufs=4))
    fin = octx.enter_context(tc.tile_pool(name="fin", bufs=3))

    # Row-tile-major: finish row-tile m's 8 scan blocks, then merge them
    # while the next row-tile's blocks stream.
    for m in range(NRT):
        vals = cand.tile([128, 64], F32, tag="VALS", name=f"VALS{m}")
        for a in range(NBLK):
            psum = mm.tile([128, 1024], F32, tag="ps", name=f"ps_{a}_{m}")
            for g in range(2):
                lhsT = xt_lhs[:, 2 * g:2 * g + 2, ts(m, 128)]
                for h in range(2):
                    nc.tensor.matmul(
                        psum[:, ts(h, 512)], lhsT,
                        xt[a][:, 2 * g:2 * g + 2, ts(h, 512)],
                        start=(g == 0), stop=(g == 1),
                        perf_mode=mybir.MatmulPerfMode.DoubleRow,
                    )
            # ACT evac with fused rounding: t = cos*2^24 + BIG (2^13 grid)
            sbt = sb.tile([128, 1024], F32, tag="sb")
            nc.scalar.activation(sbt[:], psum[:], AF.Copy,
                                 scale=EVAC_SCALE, bias=BIGMAGIC)
            # pack local column exactly: p = (t - BIG) + iota
            pkt = pk.tile([128, 1024], F32, tag="pk")
            nc.vector.scalar_tensor_tensor(
                pkt[:], in0=sbt[:], scalar=BIGMAGIC, in1=iota_f[:],
                op0=ALU.subtract, op1=ALU.add,
            )
            nc.vector.max(vals[:, a * 8:a * 8 + 8], pkt[:])

        p0 = fin.tile([128, 64], F32, tag="p0")
        nc.vector.tensor_tensor(p0[:], vals[:], off_f[:], op=ALU.add)
        pv = fin.tile([128, 24], F32, tag="pv")
        p1 = fin.tile([128, 64], F32, tag="p1")
        p2 = fin.tile([128, 64], F32, tag="p2")
        nc.vector.max(pv[:, 0:8], p0[:])
        nc.vector.match_replace(p1[:], pv[:, 0:8], p0[:], NEG)
        nc.vector.max(pv[:, 8:16], p1[:])
        nc.vector.match_replace(p2[:], pv[:, 8:16], p1[:], NEG)
        nc.vector.max(pv[:, 16:24], p2[:])
        nc.sync.dma_start(out[ts(m, 128), :], pv[:])


def _build_body(nc, x, out, cc_in, cc_out, skip_cc):
    with nc.semaphore("cc_sem") as cc_sem:
        # half 0: first 4 row-tiles -> stage -> issue its gather (no wait)
        with tile.TileContext(nc) as tc, ExitStack() as octx:
            _phase1_half(nc, tc, octx, x, cc_in[0], 0)
        if not skip_cc:
            with nc.Block() as block:
                @block.gpsimd
                def _(g: bass.BassGpSimd):
                    g.sem_clear(cc_sem)
                    g.collective_compute(
                        "AllGather", ALU.bypass,
                        replica_groups=[list(range(M))],
                        ins=[cc_in[0].ap().opt()],
                        outs=[cc_out[0].ap().opt()],
                    ).then_inc(cc_sem)

        # half 1 computes while gather 0 is in flight
        with tile.TileContext(nc) as tc, ExitStack() as octx:
            _phase1_half(nc, tc, octx, x, cc_in[1], 1)
        if not skip_cc:
            with nc.Block() as block:
                @block.gpsimd
                def _(g: bass.BassGpSimd):
                    g.collective_compute(
                        "AllGather", ALU.bypass,
                        replica_groups=[list(range(M))],
                        ins=[cc_in[1].ap().opt()],
                        outs=[cc_out[1].ap().opt()],
                    ).then_inc(cc_sem)
                    g.wait_ge(cc_sem, 2)
            nc._nrt_pseudo_barrier()

        with tile.TileContext(nc) as tc, ExitStack() as octx:
            _phase2(nc, tc, octx, out, cc_in, cc_out)


_NC = None


def _get_nc():
    global _NC
    if _NC is None:
        _NC = build_nc()
    return _NC


def run_device(x32, trace=False, **kwargs):
    """Run the SPMD kernel; returns (pv [B, 24] f32, BassKernelResults)."""
    nc = _get_nc()
    in_maps = [
        {"x": np.ascontiguousarray(x32[c * BL:(c + 1) * BL])}
        for c in range(M)
    ]
    res = run_bass_kernel_spmd(nc, in_maps, core_ids=list(range(M)),
                               trace=trace, **kwargs)
    pv = np.concatenate([res.results[c]["out"] for c in range(M)], axis=0)
    return pv, res


def decode_loss(pv, labels, k):
    """Decode packed top-24 -> (values, global column ids) -> BCE loss.

    Slot 0 is the self-match (cos = 1 dominates every row); slots 1..k
    are the true k nearest neighbours.
    """
    pv64 = pv.astype(np.float64)
    q = np.floor(pv64 / 8192.0)
    col = (pv64 - q * 8192.0).astype(np.int64)       # global column
    vhat = q / 2048.0                                 # quantized cosine
    vk = vhat[:, 1:1 + k]
    ck = col[:, 1:1 + k]
    preds = (vk + 1.0) * 0.5
    t = (labels[ck] == labels[:, None]).astype(np.float64)
    logp = np.maximum(np.log(preds), -100.0)
    log1mp = np.maximum(np.log1p(-preds), -100.0)
    loss = -(t * logp + (1.0 - t) * log1mp)
    return np.float32(loss.mean())


def kernel(batch, labels, k):
    k = int(k)
    assert 0 < k <= 23, f"kernel supports k <= 23, got {k}"
    x32 = np.asarray(batch, dtype=np.float32)
    assert x32.shape == (B, D)
    labels = np.asarray(labels)
    pv, _ = run_device(x32)
    return decode_loss(pv, labels, k)


# revision 23
# speedup vs baseline: 1.1157x; 1.1157x over previous
"""Trainium2 Bass kernel for nn_BCE_Loss (retrieval_knn).

Distributed strategy (8 NeuronCores, SPMD, AllGather):
  - Each core receives ONLY its own 1024 rows of the batch.
  - Phase 1 (per core): L2-normalize own rows (f32), scale by S=64, cast
    bf16, transpose via PE, cast fp8e4 on the PSUM->SBUF copy into an
    xT chunk [128, 4 dtiles, 1024 rows]; DMA the chunk to DRAM.
  - AllGather (NRT collective) shares the 8 fp8 chunks so every core
    holds the full xT [512, 8192] in fp8.
  - Phase 2 (per core): fp8 DoubleRow matmuls (256-deep contraction per
    instruction) compute the [1024, 8192] similarity stripe tile-by-tile
    into PSUM f32 (values = S^2 * cos); ACT evacuates each [128, 1024]
    tile with a fused magic-number rounding (t = v*(2^24/S^2) + 1.5*2^36
    snaps cos*2^24 to the 2^13 grid); DVE packs the local column exactly
    (p = (t - BIG) + iota) and takes top-8 per 1024-column block; per
    row-tile the 64 packed candidates merge via 3 x (max8 +
    match_replace) into sorted top-24 (value and global column share one
    f32). Row-tile-major order so each row-tile's merge overlaps the
    next row-tile's scans.
  - No diagonal masking: the self-match is always the global top-1
    (cos=1 vs <=0.3), so the host simply drops slot 0 and uses slots
    1..k (k <= 23).
  - Host: decode (value, column), gather labels, compute the BCE mean.
"""

from contextlib import ExitStack

import numpy as np

import concourse.bass as bass
import concourse.mybir as mybir
import concourse.tile as tile
from concourse.bass import ts
from concourse.bass_utils import run_bass_kernel_spmd
from concourse.masks import make_identity
from concourse.vector_clock import ScopedClock, VectorClock

F32 = mybir.dt.float32
BF16 = mybir.dt.bfloat16
FP8 = mybir.dt.float8e4
U32 = mybir.dt.uint32
I32 = mybir.dt.int32
AF = mybir.ActivationFunctionType
ALU = mybir.AluOpType

B, D = 8192, 512
M = 8              # cores
BL = B // M        # 1024 rows per core
NRT = BL // 128    # 8 row tiles per core
NBLK = 8           # 1024-column scan blocks
S = 64.0           # fp8 pre-scale; PSUM values are S^2 * cos
EVAC_SCALE = 16777216.0 / (S * S)   # -> cos * 2^24
BIGMAGIC = 103079215104.0  # 1.5 * 2^36: rounds cos*2^24 to multiples of 2^13
NEG = -3.0e38


# ---------------------------------------------------------------------------
# Environment workarounds: this container's walrus accepts at most ONE sem
# wait per instruction, and its runtime crashes on the explicit EventSemaphore
# butterfly barrier TileContext emits at its tail.
# ---------------------------------------------------------------------------

def _patched_drain_and_barrier(self, tick_clock, wait_clock):
    nc = self.nc
    vc = tick_clock.global_clock
    n = len(vc)
    for p in range(n):
        t = vc[p]
        if t > 0:
            pvc = VectorClock([0] * n)
            pvc.require_at_least(p, t)
            nop = nc.sync.nop()
            wait_clock.add_sem_waits(nop.ins, ScopedClock({None: pvc}))
    nc.sync.drain()
    nc._nrt_pseudo_barrier()
    assert self.sems is not None
    popped = nc._tile_sem_poison_stack.pop()
    assert popped is self._sem_poison
    nc.clear_and_free_semaphores(list(self.sems.allocated().values()))
    nc._nrt_pseudo_barrier()


tile.TileContext._drain_and_barrier = _patched_drain_and_barrier


def _split_multi_waits(nc):
    import bass_rust

    for f in nc.m.functions:
        for bb in f.blocks:
            out = []
            changed = False
            for ins in bb.instructions:
                si = ins.sync_info
                waits = list(si.on_wait) if si is not None else []
                if len(waits) > 1:
                    changed = True
                    for w in waits[:-1]:
                        nop = mybir.InstNoOp(
                            name=f"I-wsplit-{nc.next_id()}", ins=[], outs=[]
                        )
                        nop.engine = ins.engine
                        nop.sync_info = bass_rust.SyncInfo(on_wait=[w], on_update=[])
                        out.append(nop)
                    ins.sync_info = bass_rust.SyncInfo(
                        on_wait=[waits[-1]], on_update=list(si.on_update)
                    )
                out.append(ins)
            if changed:
                bb.instructions = out


# ---------------------------------------------------------------------------
# Kernel build
# ---------------------------------------------------------------------------

def build_nc(repeat=1, skip_cc=False):
    nc = bass.Bass(num_devices=M)
    x = nc.declare_dram_parameter("x", [BL, D], F32, isOutput=False)
    out = nc.declare_dram_parameter("out", [BL, 64], F32, isOutput=True)
    # per-half staging: half h covers chunk columns (own rows) h*512..h*512+512,
    # laid out d4-major: cc_in[h][:, d4*512 + c] = xT[d4*128+p, h*512+c]
    cc_in = [nc.dram_tensor(f"cc_in{h}", [128, 2048], FP8) for h in range(2)]
    cc_out = [nc.dram_tensor(f"cc_out{h}", [M * 128, 2048], FP8,
                             addr_space="Shared") for h in range(2)]
    for _rep in range(repeat):
        _build_body(nc, x, out, cc_in, cc_out, skip_cc)
    _split_multi_waits(nc)
    return nc


def _phase1_half(nc, tc, octx, x, cc_in_h, half):
    """Normalize 4 of the 8 own row-tiles, transpose, cast fp8, stage the
    half-chunk to DRAM so its AllGather can fly while the other half of
    phase 1 computes."""
    cpool = octx.enter_context(tc.tile_pool(name=f"c1{half}", bufs=1))
    ident_bf = cpool.tile([128, 128], BF16)
    make_identity(nc, ident_bf[:])

    xo_pool = octx.enter_context(tc.tile_pool(name=f"xo{half}", bufs=1))
    xt_own = xo_pool.tile([128, 4, 512], FP8)

    ld = octx.enter_context(tc.tile_pool(name=f"ld{half}", bufs=3))
    sm = octx.enter_context(tc.tile_pool(name=f"sm{half}", bufs=4))
    tpp = octx.enter_context(
        tc.tile_pool(name=f"tp{half}", bufs=2, space="PSUM"))
    for rt4 in range(4):
        rt = half * 4 + rt4
        xtile = ld.tile([128, D], F32, tag="xtile")
        nc.sync.dma_start(xtile[:], x[ts(rt, 128), :])
        sq = ld.tile([128, D], F32, tag="sq")
        ss = sm.tile([128, 1], F32, tag="ss")
        nc.scalar.activation(sq[:], xtile[:], AF.Square, accum_out=ss[:])
        nrm = sm.tile([128, 1], F32, tag="nrm")
        # nrm = sqrt(ss / S^2) = |x| / S
        nc.scalar.activation(nrm[:], ss[:], AF.Sqrt, scale=1.0 / (S * S))
        rcp = sm.tile([128, 1], F32, tag="rcp")
        nc.vector.reciprocal(rcp[:], nrm[:])       # S / |x|
        xb = ld.tile([128, D], BF16, tag="xb")
        nc.vector.tensor_scalar_mul(xb[:], xtile[:], rcp[:])
        tps = tpp.tile([128, 512], BF16, tag="tp")
        for d4 in range(4):
            nc.tensor.transpose(tps[:, ts(d4, 128)], xb[:, ts(d4, 128)],
                                ident_bf[:])
        # PSUM bf16 -> SBUF fp8 cast while laying out the half chunk
        nc.scalar.copy(
            xt_own[:, :, ts(rt4, 128)],
            tps[:].rearrange("p (d c) -> p d c", c=128),
        )
    # stage the half chunk out for its AllGather (4 x 64KB)
    for d4 in range(4):
        nc.sync.dma_start(cc_in_h[:, ts(d4, 512)], xt_own[:, d4, :])


def _phase2(nc, tc, octx, out, cc_in, cc_out):
    cpool = octx.enter_context(tc.tile_pool(name="c2", bufs=1))
    iota_i = cpool.tile([128, 1024], I32)
    nc.gpsimd.iota(iota_i[:], pattern=[[1, 1024]], base=0,
                   channel_multiplier=0)
    iota_f = cpool.tile([128, 1024], F32)
    nc.scalar.copy(iota_f[:], iota_i[:])

    xt_pool = octx.enter_context(tc.tile_pool(name="xt", bufs=1))
    xt_lhs = xt_pool.tile([128, 4, 1024], FP8, tag="xt_lhs", name="xt_lhs")
    xt = [
        xt_pool.tile([128, 4, 1024], FP8, tag=f"xt_{a}", name=f"xt_{a}")
        for a in range(NBLK)
    ]
    # own chunk (lhsT) straight from the staging bounce; gathered chunks
    # (rhs) from the collective output.
    # chunk-major fetch order: both halves of xt_lhs first, then both
    # halves of each chunk in scan order, so block a's matmuls (which
    # need both column halves) unblock after ~(a+2)*512KB instead of
    # waiting for the entire half-0 sweep.
    for h in range(2):
        nc.sync.dma_start(
            xt_lhs[:, :, h * 512:h * 512 + 512],
            cc_in[h][:, :].rearrange("p (d c) -> p d c", c=512))
    for a in range(NBLK):
        for h in range(2):
            nc.sync.dma_start(
                xt[a][:, :, h * 512:h * 512 + 512],
                cc_out[h][ts(a, 128), :].rearrange("p (d c) -> p d c",
                                                   c=512))

    mm = octx.enter_context(tc.tile_pool(name="mm", bufs=4, space="PSUM"))
    sb = octx.enter_context(tc.tile_pool(name="sb", bufs=6))
    pk = octx.enter_context(tc.tile_pool(name="pk", bufs=6))
    cand = octx.enter_context(tc.tile_pool(name="cand", bufs=4))

    # Row-tile-major: finish row-tile m's 8 scan blocks, then merge them
    # while the next row-tile's blocks stream.
    for m in range(NRT):
        vals = cand.tile([128, 64], F32, tag="VALS", name=f"VALS{m}")
        for a in range(NBLK):
            psum = mm.tile([128, 1024], F32, tag="ps", name=f"ps_{a}_{m}")
            for g in range(2):
                lhsT = xt_lhs[:, 2 * g:2 * g + 2, ts(m, 128)]
                for h in range(2):
                    nc.tensor.matmul(
                        psum[:, ts(h, 512)], lhsT,
                        xt[a][:, 2 * g:2 * g + 2, ts(h, 512)],
                        start=(g == 0), stop=(g == 1),
                        perf_mode=mybir.MatmulPerfMode.DoubleRow,
                    )
            # ACT evac with fused rounding: t = cos*2^24 + BIG (2^13 grid)
            sbt = sb.tile([128, 1024], F32, tag="sb")
            nc.scalar.activation(sbt[:], psum[:], AF.Copy,
                                 scale=EVAC_SCALE, bias=BIGMAGIC)
            # pack local column exactly: p = (t - BIG) + iota
            pkt = pk.tile([128, 1024], F32, tag="pk")
            nc.vector.scalar_tensor_tensor(
                pkt[:], in0=sbt[:], scalar=BIGMAGIC, in1=iota_f[:],
                op0=ALU.subtract, op1=ALU.add,
            )
            nc.vector.max(vals[:, a * 8:a * 8 + 8], pkt[:])

        # no on-device merge: ship all 64 per-block candidates; the host
        # adds block bases (slot position i -> block i//8), drops the
        # self-match, and selects top-k
        nc.sync.dma_start(out[ts(m, 128), :], vals[:])


def _build_body(nc, x, out, cc_in, cc_out, skip_cc):
    with nc.semaphore("cc_sem") as cc_sem:
        # half 0: first 4 row-tiles -> stage -> issue its gather (no wait)
        with tile.TileContext(nc) as tc, ExitStack() as octx:
            _phase1_half(nc, tc, octx, x, cc_in[0], 0)
        if not skip_cc:
            with nc.Block() as block:
                @block.gpsimd
                def _(g: bass.BassGpSimd):
                    g.sem_clear(cc_sem)
                    g.collective_compute(
                        "AllGather", ALU.bypass,
                        replica_groups=[list(range(M))],
                        ins=[cc_in[0].ap().opt()],
                        outs=[cc_out[0].ap().opt()],
                    ).then_inc(cc_sem)

        # half 1 computes while gather 0 is in flight
        with tile.TileContext(nc) as tc, ExitStack() as octx:
            _phase1_half(nc, tc, octx, x, cc_in[1], 1)
        if not skip_cc:
            with nc.Block() as block:
                @block.gpsimd
                def _(g: bass.BassGpSimd):
                    g.collective_compute(
                        "AllGather", ALU.bypass,
                        replica_groups=[list(range(M))],
                        ins=[cc_in[1].ap().opt()],
                        outs=[cc_out[1].ap().opt()],
                    ).then_inc(cc_sem)
                    g.wait_ge(cc_sem, 2)
            nc._nrt_pseudo_barrier()

        with tile.TileContext(nc) as tc, ExitStack() as octx:
            _phase2(nc, tc, octx, out, cc_in, cc_out)


_NC = None


def _get_nc():
    global _NC
    if _NC is None:
        _NC = build_nc()
    return _NC


def run_device(x32, trace=False, **kwargs):
    """Run the SPMD kernel; returns (pv [B, 24] f32, BassKernelResults)."""
    nc = _get_nc()
    in_maps = [
        {"x": np.ascontiguousarray(x32[c * BL:(c + 1) * BL])}
        for c in range(M)
    ]
    res = run_bass_kernel_spmd(nc, in_maps, core_ids=list(range(M)),
                               trace=trace, **kwargs)
    pv = np.concatenate([res.results[c]["out"] for c in range(M)], axis=0)
    return pv, res


def decode_loss(pv, labels, k):
    """Decode the 64 packed per-block candidates -> top-k -> BCE loss.

    pv[r, i] packs (value, local column) for scan block i // 8; the self
    match (cos = 1 dominates every row) is masked out, then the k best
    by packed value are the true k nearest neighbours.
    """
    Bn = pv.shape[0]
    pv64 = pv.astype(np.float64)
    q = np.floor(pv64 / 8192.0)
    col = (pv64 - q * 8192.0).astype(np.int64)
    col = col + (np.arange(64)[None, :] // 8) * 1024  # block base -> global
    vhat = q / 2048.0                                 # quantized cosine
    # sorted slot 0 is always the self-match (packed value ~2^24 vs
    # <=0.36*2^24 for real neighbours); drop it positionally - its packed
    # column bits may be off by one (p >= 2^24 loses the odd bit), so a
    # col == row mask is NOT reliable
    sel = np.argsort(-pv64, axis=1)[:, 1:1 + k]       # top-k by (value, col)
    rows = np.arange(Bn)[:, None]
    vk = vhat[rows, sel]
    ck = col[rows, sel]
    preds = (vk + 1.0) * 0.5
    t = (labels[ck] == labels[:, None]).astype(np.float64)
    logp = np.maximum(np.log(preds), -100.0)
    log1mp = np.maximum(np.log1p(-preds), -100.0)
    loss = -(t * logp + (1.0 - t) * log1mp)
    return np.float32(loss.mean())


def kernel(batch, labels, k):
    k = int(k)
    assert 0 < k <= 55, f"kernel supports k <= 55, got {k}"
    x32 = np.asarray(batch, dtype=np.float32)
    assert x32.shape == (B, D)
    labels = np.asarray(labels)
    pv, _ = run_device(x32)
    return decode_loss(pv, labels, k)
